# revision 4
# baseline (speedup 1.0000x reference)
"""Trainium2 Bass kernel for nn_CGWeight (CG-weighted bilinear message passing).

out[e, k] = sum_c w_c * einsum('ijk,ei,ej->ek', cg_c, a_{l1(c)}, h_{l2(c)})
          = x_e^T B_k y_e,  x = concat(a0,a1,a2), y = concat(h0,h1,h2) (9 feats).

Strategy (data-parallel over E across 8 cores):
  B (9x9x3) is CP-decomposed EXACTLY at rank R=14 on host (ALS + LM polish;
  generic rank of a 9x9x3 tensor is 14):
      B[i,j,k] = sum_r U[i,r] V[j,r] W[k,r]
      out[e,k] = sum_r W[k,r] * (U^T x_e)_r * (V^T y_e)_r
  Per core, edges are packed slot-interleaved: S=9 edge slots x 9 features
  = 81 partitions for inputs; S x R = 126 partitions for intermediates.
  Per 512-column tile (4608 edges):
      p = Ublk^T x_tile   (PE, f16 in / f32 PSUM)          1 stream
      q = Vblk^T y_tile   (PE, into paired 2-bank PSUM)    1 stream
      qs = copy(q-pair)   (ACT, PSUM->SBUF f32, 2 tiles/op)
      t = p * qs          (DVE, f32r out)
      out += Wblk_j^T t   (PE, 4 tiles accumulate into one PSUM bank via
                           zero-padded weight variants j=0..3)  1 stream
      osb = f16(out)      (ACT, 1 copy per 4 tiles)
  PE does 3x512-cycle streams per tile; DVE one tensor_tensor; ACT ~0.64
  ops/tile; DMA 36 B/edge in + ~7 B/edge out -> all engines balanced near
  the ~358 GB/s memory roofline.
"""
import numpy as np

import concourse.bass as bass
import concourse.mybir as mybir
from concourse import tile
from concourse.bass_utils import run_bass_kernel_spmd

E = 3_200_000
N_CORES = 8
E_CORE = E // N_CORES          # 400_000
R = 14                         # CP rank (exact for generic 9x9x3)
S = 9                          # edge slots per matmul column
P_X = 9 * S                    # 81 input partitions
P_T = R * S                    # 126 intermediate partitions
TILE_N = 512
T = 88                         # tiles per core: 9*88*512 = 405504 >= 400000
NBLK = T * TILE_N              # 45056 edges per slot
E_PAD = S * NBLK               # 405504
CHUNK = 11                     # tiles per input DMA (81x5632 f16 = 912 KiB)
N_CHUNK = T // CHUNK           # 8
OG = 4                         # tiles accumulated per output PSUM bank
G2 = 2                         # out groups per SBUF staging tile / DMA
N_ODMA = T // (OG * G2)        # 11
P_O = 3 * 32 + 27              # 123 rows: group j at partitions 32j..32j+26

COMBOS = [(0, 1), (1, 0), (1, 1), (1, 2), (2, 1), (2, 2)]
OFF = {0: 0, 1: 1, 2: 4}
DIM = {0: 1, 1: 3, 2: 5}

_F32 = mybir.dt.float32
_F32R = mybir.dt.float32r
_F16 = mybir.dt.float16


def _split_multi_waits(nc, max_waits=1):
    """walrus CoreV3 setupSyncWait only accepts one sync-wait per
    instruction; hoist extra waits onto same-engine NoOps placed before."""
    ctr = 0
    for fn in nc.m.functions:
        for blk in fn.blocks:
            out = []
            changed = False
            for ins in blk.instructions:
                si = getattr(ins, "sync_info", None)
                waits = list(si.on_wait) if si is not None else []
                if len(waits) > max_waits:
                    changed = True
                    keep = waits[-max_waits:]
                    for w in waits[:-max_waits]:
                        ctr += 1
                        out.append(mybir.InstNoOp(
                            name=f"I-waitsplit-{ctr}",
                            engine=ins.engine,
                            ins=[], outs=[],
                            sync_info=mybir.SyncInfo(on_wait=[w], on_update=[]),
                        ))
                    ins.sync_info = mybir.SyncInfo(
                        on_wait=keep, on_update=list(si.on_update))
                out.append(ins)
            if changed:
                blk.instructions = out
    return nc


def _build_nc():
    nc = bass.Bass()
    xt_d = nc.dram_tensor("xt", [P_X, NBLK], _F16, kind="ExternalInput")
    yt_d = nc.dram_tensor("yt", [P_X, NBLK], _F16, kind="ExternalInput")
    ublk_d = nc.dram_tensor("ublk", [P_X, P_T], _F16, kind="ExternalInput")
    vblk_d = nc.dram_tensor("vblk", [P_X, P_T], _F16, kind="ExternalInput")
    # 4 zero-padded variants: variant j maps t -> rows 32j..32j+26 of [123]
    wblk_d = nc.dram_tensor("wblk", [P_T, OG * P_O], _F32R, kind="ExternalInput")
    o_d = nc.dram_tensor("o", [N_ODMA * P_O, G2 * TILE_N], _F16,
                         kind="ExternalOutput")

    with tile.TileContext(nc) as tc:
        with (
            tc.tile_pool(name="consts", bufs=1) as cpool,
            tc.tile_pool(name="inx", bufs=2) as xpool,
            tc.tile_pool(name="iny", bufs=2) as ypool,
            tc.tile_pool(name="qstage", bufs=2) as qspool,
            tc.tile_pool(name="tprod", bufs=3) as tpool,
            tc.tile_pool(name="outs", bufs=2) as opool,
            tc.tile_pool(name="psP", bufs=2, space="PSUM") as psP,
            tc.tile_pool(name="psQ", bufs=2, space="PSUM") as psQ,
            tc.tile_pool(name="psO", bufs=2, space="PSUM") as psO,
        ):
            ublk = cpool.tile([P_X, P_T], _F16, tag="ublk")
            vblk = cpool.tile([P_X, P_T], _F16, tag="vblk")
            wblk = cpool.tile([P_T, OG * P_O], _F32R, tag="wblk")
            nc.sync.dma_start(ublk[:], ublk_d[:])
            nc.sync.dma_start(vblk[:], vblk_d[:])
            nc.sync.dma_start(wblk[:], wblk_d[:])

            qq = None
            oo = None
            osb = None
            pp_prev = None
            # chunk tiles: per-pair pipeline (stage q for 2 tiles at once)
            for ch in range(N_CHUNK):
                c0 = ch * CHUNK * TILE_N
                c1 = (ch + 1) * CHUNK * TILE_N
                xc = xpool.tile([P_X, CHUNK * TILE_N], _F16, tag="xc")
                yc = ypool.tile([P_X, CHUNK * TILE_N], _F16, tag="yc")
                nc.sync.dma_start(xc[:], xt_d[:, c0:c1])
                nc.sync.dma_start(yc[:], yt_d[:, c0:c1])
                for ti in range(CHUNK):
                    m = ch * CHUNK + ti          # global tile index
                    co = ti * TILE_N
                    par = m % 2
                    pp = psP.tile([P_T, TILE_N], _F32, tag="p")
                    nc.tensor.matmul(pp[:], ublk[:], xc[:, co:co + TILE_N],
                                     start=True, stop=True)
                    if par == 0:
                        qq = psQ.tile([P_T, 2 * TILE_N], _F32, tag="q")
                    nc.tensor.matmul(qq[:, par * TILE_N:(par + 1) * TILE_N],
                                     vblk[:], yc[:, co:co + TILE_N],
                                     start=True, stop=True)
                    if par == 0:
                        pp_prev = pp
                        continue
                    qs = qspool.tile([P_T, 2 * TILE_N], _F32, tag="qs")
                    nc.scalar.copy(qs[:], qq[:])
                    for mm, ppx in ((m - 1, pp_prev), (m, pp)):
                        half = mm % 2
                        t = tpool.tile([P_T, TILE_N], _F32R, tag="t")
                        nc.vector.tensor_mul(
                            t[:], ppx[:],
                            qs[:, half * TILE_N:(half + 1) * TILE_N])
                        j = mm % OG
                        if j == 0:
                            oo = psO.tile([P_O, TILE_N], _F32, tag="o")
                        nc.tensor.matmul(
                            oo[:], wblk[:, j * P_O:(j + 1) * P_O],
                            t[:], start=(j == 0), stop=(j == OG - 1))
                        if j == OG - 1:
                            g = mm // OG
                            h = g % G2
                            if h == 0:
                                osb = opool.tile([P_O, G2 * TILE_N], _F16,
                                                 tag="osb")
                            nc.scalar.copy(
                                osb[:, h * TILE_N:(h + 1) * TILE_N], oo[:])
                            if h == G2 - 1:
                                g2 = g // G2
                                nc.sync.dma_start(
                                    o_d[g2 * P_O:(g2 + 1) * P_O, :], osb[:])

    _split_multi_waits(nc)
    return nc


_NC_CACHE = None


def _get_nc():
    global _NC_CACHE
    if _NC_CACHE is None:
        _NC_CACHE = _build_nc()
    return _NC_CACHE


def _cp_decompose(B, rank, seeds=10):
    """Exact-ish CP decomposition of B [9,9,3] via ALS warmup + damped
    Gauss-Newton (LM). Returns U [9,R], V [9,R], W [3,R]."""
    I, J, K = B.shape
    nB = np.linalg.norm(B)
    B1 = B.reshape(I, J * K)
    B2 = B.transpose(1, 0, 2).reshape(J, I * K)
    B3 = B.transpose(2, 0, 1).reshape(K, I * J)
    EI, EJ, EK = np.eye(I), np.eye(J), np.eye(K)

    def bhat(U, V, W):
        return np.einsum('ir,jr,kr->ijk', U, V, W)

    best = None
    for seed in range(seeds):
        rng = np.random.default_rng(seed)
        U = rng.standard_normal((I, rank)) * 0.5
        V = rng.standard_normal((J, rank)) * 0.5
        W = rng.standard_normal((K, rank)) * 0.5
        for _ in range(200):
            KR = np.einsum('jr,kr->jkr', V, W).reshape(J * K, rank)
            U = B1 @ KR @ np.linalg.pinv(KR.T @ KR, rcond=1e-14)
            KR = np.einsum('ir,kr->ikr', U, W).reshape(I * K, rank)
            V = B2 @ KR @ np.linalg.pinv(KR.T @ KR, rcond=1e-14)
            KR = np.einsum('ir,jr->ijr', U, V).reshape(I * J, rank)
            W = B3 @ KR @ np.linalg.pinv(KR.T @ KR, rcond=1e-14)
        lam = 1e-4
        r = (bhat(U, V, W) - B).ravel()
        cost = r @ r
        n = (I + J + K) * rank
        for _ in range(400):
            vw = np.einsum('jr,kr->jkr', V, W)
            uw = np.einsum('ir,kr->ikr', U, W)
            uv = np.einsum('ir,jr->ijr', U, V)
            JU = np.einsum('ix,jkr->ijkxr', EI, vw).reshape(I * J * K, I * rank)
            JV = np.einsum('jx,ikr->ijkxr', EJ, uw).reshape(I * J * K, J * rank)
            JW = np.einsum('kx,ijr->ijkxr', EK, uv).reshape(I * J * K, K * rank)
            Jm = np.concatenate([JU, JV, JW], axis=1)
            g = Jm.T @ r
            H = Jm.T @ Jm
            ok = False
            for _ in range(30):
                delta = np.linalg.solve(H + lam * np.eye(n), -g)
                Un = U + delta[:I * rank].reshape(I, rank)
                Vn = V + delta[I * rank:(I + J) * rank].reshape(J, rank)
                Wn = W + delta[(I + J) * rank:].reshape(K, rank)
                rn = (bhat(Un, Vn, Wn) - B).ravel()
                cn = rn @ rn
                if np.isfinite(cn) and cn < cost:
                    U, V, W, r, cost = Un, Vn, Wn, rn, cn
                    lam = max(lam * 0.3, 1e-12)
                    ok = True
                    break
                lam *= 10
            if not ok or cost < (1e-10 * nB) ** 2:
                break
        res = np.sqrt(cost) / nB
        if best is None or res < best[3]:
            best = (U, V, W, res)
        if best[3] < 1e-8:
            break
    U, V, W, res = best
    # balance column norms (fp16 range/precision hygiene)
    for r_ in range(rank):
        su, sv = np.linalg.norm(U[:, r_]), np.linalg.norm(V[:, r_])
        s = np.sqrt(su * sv)
        if su > 0 and sv > 0:
            U[:, r_] *= s / su
            V[:, r_] *= s / sv
            W[:, r_] *= (su * sv) / (s * s)
    return U, V, W, res


def _make_consts(weight, cgd):
    B = np.zeros((9, 9, 3), np.float64)
    for ci, (l1, l2) in enumerate(COMBOS):
        B[OFF[l1]:OFF[l1] + DIM[l1], OFF[l2]:OFF[l2] + DIM[l2], :] += (
            weight[ci] * cgd[(l1, l2)].astype(np.float64))
    U, V, W, res = _cp_decompose(B, R)
    # slot-block-diagonal expansions
    ublk = np.zeros((P_X, P_T), np.float32)
    vblk = np.zeros((P_X, P_T), np.float32)
    for s in range(S):
        ublk[s * 9:(s + 1) * 9, s * R:(s + 1) * R] = U
        vblk[s * 9:(s + 1) * 9, s * R:(s + 1) * R] = V
    # wblk variant j: [(s,r), 32j + s*3 + k] = W[k,r]
    wblk = np.zeros((P_T, OG * P_O), np.float32)
    for j in range(OG):
        for s in range(S):
            wblk[s * R:(s + 1) * R,
                 j * P_O + 32 * j + s * 3:j * P_O + 32 * j + s * 3 + 3] = W.T
    return ublk, vblk, wblk, res


def _feature_major(a0, a1, a2, lo, hi):
    """[81, NBLK] slot-interleaved feature-major slice of concat(a0,a1,a2)
    rows lo:hi. Row s*9+i, col n = feature i of edge s*NBLK + n (rel lo)."""
    f = np.zeros((9, E_PAD), np.float32)
    f[0, :hi - lo] = a0[lo:hi, 0]
    f[1:4, :hi - lo] = a1[lo:hi].T
    f[4:9, :hi - lo] = a2[lo:hi].T
    return np.ascontiguousarray(
        f.reshape(9, S, NBLK).transpose(1, 0, 2).reshape(P_X, NBLK))


def _run_spmd(inputs, trace=False):
    a0 = np.asarray(inputs["a0"], np.float32)
    a1 = np.asarray(inputs["a1"], np.float32)
    a2 = np.asarray(inputs["a2"], np.float32)
    h0 = np.asarray(inputs["h0"], np.float32)
    h1 = np.asarray(inputs["h1"], np.float32)
    h2 = np.asarray(inputs["h2"], np.float32)
    weight = np.asarray(inputs["weight"], np.float32)
    cgd = {(l1, l2): np.asarray(inputs[f"cg{l1}{l2}"], np.float32)
           for (l1, l2) in COMBOS}

    ublk, vblk, wblk, res = _make_consts(weight, cgd)
    ublk16 = ublk.astype(np.float16)
    vblk16 = vblk.astype(np.float16)
    in_maps = []
    for c in range(N_CORES):
        lo, hi = c * E_CORE, (c + 1) * E_CORE
        in_maps.append({
            "xt": _feature_major(a0, a1, a2, lo, hi).astype(np.float16),
            "yt": _feature_major(h0, h1, h2, lo, hi).astype(np.float16),
            "ublk": ublk16, "vblk": vblk16, "wblk": wblk,
        })

    nc = _get_nc()
    br = run_bass_kernel_spmd(nc, in_maps, list(range(N_CORES)), trace=trace)

    out = np.empty((E, 3), np.float32)
    for c in range(N_CORES):
        O = br.results[c]["o"]                     # [11*123, 1024] f16
        O = O.reshape(N_ODMA, P_O, G2, TILE_N)     # [g2, row, h, n]
        # row = 32j + s*3 + k for tile m = (g2*G2 + h)*OG + j
        dec = np.empty((S, N_ODMA, G2, OG, TILE_N, 3), np.float32)
        for j in range(OG):
            blk = O[:, 32 * j:32 * j + 27, :, :].astype(np.float32)
            # [g2, (s,k), h, n] -> [s, g2, h, n, k]
            dec[:, :, :, j] = blk.reshape(N_ODMA, S, 3, G2, TILE_N).transpose(
                1, 0, 3, 4, 2)
        # edge = s*NBLK + ((g2*G2 + h)*OG + j)*512 + n
        out[c * E_CORE:(c + 1) * E_CORE] = dec.reshape(E_PAD, 3)[:E_CORE]
    return out, br


def kernel(**inputs):
    out, _ = _run_spmd(inputs, trace=False)
    return out
